# revision 13
# baseline (speedup 1.0000x reference)
"""Top-1 MoE layer (BASE-layer style) on 8 Trainium2 NeuronCores.

Expert-parallel: core e holds expert e's weights. The host computes the
top-1 gating assignment (a tiny [T,E] matmul + argmax), dispatches each
expert's tokens to its core (this realizes the All2All of the reference
module), each core runs LN -> FF1 -> ReLU -> FF2 over its token batch,
and the host adds the residual + b2 and scatters the per-expert outputs
back into token order.

Per-core device kernel (capacity C tokens, D=1024, F=4096), bf16 matmul:
  - LN in token-major layout via bn_stats/bn_aggr
  - PE-transpose of xn into D-major, LN affine fused into the eviction
  - MM1: hT[f,t] = relu(W1.T @ xnT + b1); d-major weight-reuse order so
    every LDWEIGHTS hides behind a >=128-wide matmul; moving chunks
    (448, 128) so chunk-1 matmuls still cover LDWEIGHTS of the next tile
  - MM2 in d-major: y[d,t] = W2tile.T @ hT, no padded token tile
  - PSUM evictions round-robined across ACT / DVE / GPSIMD so no single
    engine gates the PE
  - residual add + b2 + final dtype handled on the host; device output
    is y_ff in bf16 (halves output DMA)
Input x and W1/W2 DMAs are spread over four engine queues with x first
so LN starts as early as the queues can deliver.
"""

import math

import numpy as np
import ml_dtypes

import concourse.bass as bass
import concourse.tile as tile
from concourse import bacc, mybir
from concourse.bass_utils import run_bass_kernel_spmd
from concourse.masks import make_identity

E = 8
D = 1024
F = 4096
LN_EPS = 1e-5
P = 128
F32 = mybir.dt.float32
BF16 = mybir.dt.bfloat16

DO = D // P      # 8 d-tiles
FO = F // P      # 32 f-tiles
NC1 = 16         # W1 macro chunks (2 f-tiles each)
NF1 = FO // NC1  # f-tiles per W1 chunk

# set by test.py to get a profile
TRACE = False
TRACE_DIR = None
LAST_EXEC_TIME_NS = None
LAST_RESULTS = None

_program_cache = {}


def _mm_chunks(C):
    """Moving-dim chunks: first up to 448 wide, rest 128-wide (<=512 so a
    chunk fits one PSUM bank; 128 tails keep the next LDWEIGHTS hidden)."""
    if C <= 512:
        return [(0, C)]
    out = [(0, 448)]
    t = 448
    while t < C:
        w = min(128, C - t)
        out.append((t, w))
        t += w
    return out


def build_program(C: int):
    """SPMD per-core Bass program for token capacity C (multiple of 64)."""
    assert C % 64 == 0
    NT = math.ceil(C / P)
    subtiles = []
    t = 0
    while t < C:
        w = min(P, C - t)
        subtiles.append((t, w))
        t += w
    chunks = _mm_chunks(C)

    nc = bacc.Bacc(None, target_bir_lowering=False, debug=False)

    # host-prearranged layouts (see kernel() below)
    xe_d = nc.dram_tensor("xe", [P, NT, D], BF16, kind="ExternalInput")
    w1_d = nc.dram_tensor("w1", [NC1, P, NF1, DO, P], BF16, kind="ExternalInput")
    w2_d = nc.dram_tensor("w2", [4, P, FO // 4, DO, P], BF16, kind="ExternalInput")
    b1_d = nc.dram_tensor("b1", [P, FO], F32, kind="ExternalInput")
    g_d = nc.dram_tensor("ln_g", [P, DO], F32, kind="ExternalInput")
    bb_d = nc.dram_tensor("ln_b", [P, DO], F32, kind="ExternalInput")
    ye_d = nc.dram_tensor("ye", [4, P, DO // 4, C], BF16, kind="ExternalOutput")

    with tile.TileContext(nc) as tc:
        with (
            tc.tile_pool(name="consts", bufs=1) as consts,
            tc.tile_pool(name="w2p", bufs=1) as w2p,
            tc.tile_pool(name="w1p", bufs=3) as w1p,
            tc.tile_pool(name="xp", bufs=1) as xp,
            tc.tile_pool(name="xnp", bufs=1) as xnp,
            tc.tile_pool(name="xtp", bufs=1) as xtp,
            tc.tile_pool(name="hp", bufs=1) as hp,
            tc.tile_pool(name="yp", bufs=1) as yp,
            tc.tile_pool(name="stat", bufs=6) as stat,
            tc.tile_pool(name="pst", bufs=2, space="PSUM") as pst,
            tc.tile_pool(name="psA", bufs=6, space="PSUM") as psA,
        ):
            # ---- input DMAs ----
            # x subtiles (bf16) first on the sync queue so LN can start as
            # soon as the DMA rings spin up; weights ride the other queues
            # x: partition-major so every DMA line is the full NT*D row
            # (10KB): big packets = fast queue; split halves across the two
            # earliest queues so they move in parallel
            x_t = xp.tile([P, NT, D], BF16, tag="x")
            nc.sync.dma_start(out=x_t[:64], in_=xe_d[:64])
            g_t = consts.tile([P, DO], F32)
            nc.scalar.dma_start(out=g_t, in_=g_d[:])
            bb_t = consts.tile([P, DO], F32)
            nc.scalar.dma_start(out=bb_t, in_=bb_d[:])
            nc.scalar.dma_start(out=x_t[64:], in_=xe_d[64:])
            b1_t = consts.tile([P, FO], F32)
            nc.gpsimd.dma_start(out=b1_t, in_=b1_d[:])

            # W2 resident: halves split over the scalar (behind x) and gpsimd
            # (behind W1) queues; both land well before MM2 needs them
            w2_t = w2p.tile([P, FO, DO, P], BF16)
            for h in range(2):
                nc.scalar.dma_start(out=w2_t[:, h * 8:(h + 1) * 8], in_=w2_d[h])

            ident = consts.tile([P, P], BF16)
            make_identity(nc, ident)
            eps_t = consts.tile([P, 1], F32)
            nc.vector.memset(eps_t, LN_EPS)

            # ---- LN: stats + normalize on DVE, rsqrt on ACT/DVE ----
            xn_t = xnp.tile([P, NT, D], BF16, tag="xn")
            for i, (ss, sw) in enumerate(subtiles):
                st = stat.tile([P, 2, 6], F32, tag="st")
                for h in range(2):
                    nc.vector.bn_stats(
                        out=st[:sw, h, :], in_=x_t[:sw, i, h * 512:(h + 1) * 512]
                    )
                mv = stat.tile([P, 2], F32, tag="mv")
                nc.vector.bn_aggr(out=mv[:sw], in_=st[:sw])
                rstd = stat.tile([P, 1], F32, tag="rstd")
                nc.scalar.activation(
                    out=rstd[:sw], in_=mv[:sw, 1:2],
                    func=mybir.ActivationFunctionType.Sqrt,
                    bias=eps_t[:sw], scale=1.0,
                )
                nc.vector.reciprocal(out=rstd[:sw], in_=rstd[:sw])
                # xn = (x - mean) * rstd   (cast to bf16 on write)
                nc.vector.tensor_scalar(
                    out=xn_t[:sw, i, :], in0=x_t[:sw, i, :],
                    scalar1=mv[:sw, 0:1], scalar2=rstd[:sw],
                    op0=mybir.AluOpType.subtract, op1=mybir.AluOpType.mult,
                )

            # eviction engines, round-robined ACT/DVE (GPSIMD cannot
            # read PSUM) so neither gates the PE
            def evict_affine(k, out, ps, do):
                # out = ps * g[do] + b[do]  (per-partition scalars, d-major)
                if k % 2 == 0:
                    nc.scalar.activation(
                        out=out, in_=ps,
                        func=mybir.ActivationFunctionType.Identity,
                        bias=bb_t[:, do:do + 1], scale=g_t[:, do:do + 1],
                    )
                else:
                    nc.vector.tensor_scalar(
                        out=out, in0=ps,
                        scalar1=g_t[:, do:do + 1], scalar2=bb_t[:, do:do + 1],
                        op0=mybir.AluOpType.mult, op1=mybir.AluOpType.add,
                    )

            def evict_relu(k, out, ps, fo):
                # out = relu(ps + b1[fo])
                if k % 2 == 0:
                    nc.scalar.activation(
                        out=out, in_=ps,
                        func=mybir.ActivationFunctionType.Relu,
                        bias=b1_t[:, fo:fo + 1], scale=1.0,
                    )
                else:
                    nc.vector.tensor_scalar(
                        out=out, in0=ps,
                        scalar1=b1_t[:, fo:fo + 1], scalar2=0.0,
                        op0=mybir.AluOpType.add, op1=mybir.AluOpType.max,
                    )

            def evict_copy(k, out, ps):
                if k % 2 == 0:
                    nc.scalar.activation(
                        out=out, in_=ps,
                        func=mybir.ActivationFunctionType.Identity,
                    )
                else:
                    nc.vector.tensor_scalar(
                        out=out, in0=ps, scalar1=1.0, scalar2=None,
                        op0=mybir.AluOpType.mult,
                    )

            # ---- transpose xn -> xnT [d_in, do, tok], LN affine fused ----
            xnT = xtp.tile([P, DO, C], BF16, tag="xnT")
            for i, (ss, sw) in enumerate(subtiles):
                for do in range(DO):
                    ps = pst.tile([P, P], BF16, tag="pst")
                    nc.tensor.transpose(
                        ps[:, :sw], xn_t[:sw, i, do * P:(do + 1) * P], ident[:sw, :sw]
                    )
                    evict_affine(i * DO + do, xnT[:, do, ss:ss + sw], ps[:, :sw], do)

            # ---- MM1: hT[f, t] = relu(W1.T @ xnT + b1) ----
            # d-major weight reuse: one stationary tile serves every moving
            # chunk before the PE moves on.
            hT = hp.tile([P, FO, C], BF16, tag="hT")
            for c in range(NC1):
                w1c = w1p.tile([P, NF1, DO, P], BF16, tag="w1c")
                nc.gpsimd.dma_start(out=w1c, in_=w1_d[c])
                for f in range(NF1):
                    fo = c * NF1 + f
                    phs = [
                        psA.tile([P, 512], F32, tag="pbig", name="pbig")
                        for _ in chunks
                    ]
                    for do in range(DO):
                        for ph, (cs, cw) in zip(phs, chunks):
                            nc.tensor.matmul(
                                ph[:, :cw],
                                w1c[:, f, do, :],
                                xnT[:, do, cs:cs + cw],
                                start=(do == 0), stop=(do == DO - 1),
                            )
                    for j, (ph, (cs, cw)) in enumerate(zip(phs, chunks)):
                        evict_relu(fo + j, hT[:, fo, cs:cs + cw], ph[:, :cw], fo)

            for h in range(2, 4):
                nc.gpsimd.dma_start(out=w2_t[:, h * 8:(h + 1) * 8], in_=w2_d[h])

            # ---- MM2 (d-major): y[d_in, do, t] = sum_fo W2[fo,do].T @ hT[fo] ----
            y_t = yp.tile([P, DO, C], BF16, tag="y")
            for do in range(DO):
                pds = [
                    psA.tile([P, 512], F32, tag="pbig", name="pbig")
                    for _ in chunks
                ]
                for fo in range(FO):
                    for pd, (cs, cw) in zip(pds, chunks):
                        nc.tensor.matmul(
                            pd[:, :cw],
                            w2_t[:, fo, do, :],
                            hT[:, fo, cs:cs + cw],
                            start=(fo == 0), stop=(fo == FO - 1),
                        )
                for j, (pd, (cs, cw)) in enumerate(zip(pds, chunks)):
                    evict_copy(do + j, y_t[:, do, cs:cs + cw], pd[:, :cw])
                if do % (DO // 4) == DO // 4 - 1:
                    h = do // (DO // 4)
                    sl = slice(h * (DO // 4), (h + 1) * (DO // 4))
                    if h < 3:
                        nc.sync.dma_start(out=ye_d[h], in_=y_t[:, sl, :])
                    else:
                        nc.sync.dma_start(out=ye_d[h, :64], in_=y_t[:64, sl, :])
                        nc.scalar.dma_start(out=ye_d[h, 64:], in_=y_t[64:, sl, :])

    nc.compile()
    if not nc.is_finalized():
        nc.finalize()
    return nc


def kernel(input_features, centroids, ln_g, ln_b, W1, b1, W2, b2):
    global LAST_EXEC_TIME_NS, LAST_RESULTS
    x = np.asarray(input_features)
    S, B, _ = x.shape
    xt = np.ascontiguousarray(np.swapaxes(x, 0, 1).reshape(-1, D))  # [T, D]
    T = xt.shape[0]

    # host gating: tiny [T,E] matmul + argmax (same fp32 math / first-max
    # tie-break as the reference)
    logits = xt @ np.asarray(centroids, np.float32).T
    assign = np.argmax(logits, axis=-1)
    order = [np.nonzero(assign == e)[0] for e in range(E)]
    counts = [len(o) for o in order]
    C = max(64, int(math.ceil(max(counts) / 64)) * 64)
    NT = math.ceil(C / P)

    bf = ml_dtypes.bfloat16
    # pre-layouts: every DMA line is multi-KB contiguous per partition
    # w1: [D,F] -> [din(P), c, f4, do, fin];  w2: [F,D] -> [fin(P), fo, do, dfree]
    W1p = np.ascontiguousarray(
        np.asarray(W1).astype(bf)
        .reshape(E, DO, P, NC1, NF1, P).transpose(0, 3, 2, 4, 1, 5)
    )
    W2p = np.ascontiguousarray(
        np.asarray(W2).astype(bf).reshape(E, 4, FO // 4, P, DO, P)
        .transpose(0, 1, 3, 2, 4, 5)
    )
    b1p = np.ascontiguousarray(
        np.asarray(b1, np.float32).reshape(E, FO, P).transpose(0, 2, 1)
    )
    gp = np.ascontiguousarray(
        np.asarray(ln_g, np.float32).reshape(E, DO, P).transpose(0, 2, 1)
    )
    bbp = np.ascontiguousarray(
        np.asarray(ln_b, np.float32).reshape(E, DO, P).transpose(0, 2, 1)
    )

    in_maps = []
    for e in range(E):
        xe = np.zeros((NT * P, D), bf)
        xe[:counts[e]] = xt[order[e]].astype(bf)
        # token (nt*128 + p) lives at [p, nt, :]
        xe = np.ascontiguousarray(xe.reshape(NT, P, D).transpose(1, 0, 2))
        in_maps.append({
            "xe": xe,
            "w1": W1p[e],
            "w2": W2p[e],
            "b1": b1p[e],
            "ln_g": gp[e],
            "ln_b": bbp[e],
        })

    if C not in _program_cache:
        _program_cache[C] = build_program(C)
    nc = _program_cache[C]

    kw = {}
    if TRACE:
        kw = {"trace": True, "tmpdir": TRACE_DIR}
    res = run_bass_kernel_spmd(nc, in_maps, list(range(E)), **kw)
    LAST_EXEC_TIME_NS = res.exec_time_ns
    LAST_RESULTS = res

    b2f = np.asarray(b2, np.float32)
    out = np.empty((T, D), np.float32)
    for e in range(E):
        ye = np.asarray(res.results[e]["ye"])        # [4, P, DO//4, C] bf16
        yff = np.ascontiguousarray(ye.transpose(3, 0, 2, 1)).reshape(C, D)
        out[order[e]] = (
            xt[order[e]] + yff[: counts[e]].astype(np.float32) + b2f[e]
        )
    return np.ascontiguousarray(np.swapaxes(out.reshape(B, S, D), 0, 1))
